# revision 3
# baseline (speedup 1.0000x reference)
"""Multi-head attention (QKV proj + RoPE + softmax attention + o-proj) on 8
Trainium2 NeuronCores.

Sharding: data-parallel over batch (B=2) x tensor-parallel over heads
(16 heads -> 4 groups of 4). Core c handles batch c//4, heads 4*(c%4)..+4.
qkv_proj is column-parallel, o_proj row-parallel; each core returns a
partial o-proj output and the host sums the 4 partials per batch.

All matmuls run in fp16 (full PE speed, ~8x better precision than bf16;
all values here are well inside fp16 range) with fp32 PSUM accumulation.

Layout notes (per core):
 - x is fed transposed: xT [HID, N] so QKV matmuls contract over hid on
   partitions.
 - q,k are produced as [dh, tok] (head dim on partitions), which feeds the
   transposed-scores matmul S^T[k_tok, q_tok] = k^T q directly.
 - softmax runs on S^T: exp on the scalar engine (scale folded in), the
   denominator comes from an all-ones [128,128] stationary matmul
   (every output partition = sum over k), PV accumulates out^T [dh, q].
 - v is produced as [tok, dh*4heads] so it directly serves as the PV
   stationary operand.
"""

import sys

if "/opt/trn_rl_repo" not in sys.path:
    sys.path.insert(0, "/opt/trn_rl_repo")

import numpy as np

import concourse.bass as bass
import concourse.mybir as mybir
import concourse.tile as tile
from concourse import bacc

B, N, HID, H = 2, 2048, 2048, 16
DH = 128
HPC = 4  # heads per core
P = 128
F16 = mybir.dt.float16
F32 = mybir.dt.float32
SCALE = 1.0 / float(np.sqrt(DH))

_NC_CACHE = [None]


def build_nc():
    nc = bacc.Bacc(None, target_bir_lowering=False)

    xT = nc.dram_tensor("xT", [HID, N], F16, kind="ExternalInput")
    wqkT = nc.dram_tensor("wqkT", [HID, 2 * HPC * DH], F16, kind="ExternalInput")
    wvT = nc.dram_tensor("wvT", [HID, HPC * DH], F16, kind="ExternalInput")
    woT = nc.dram_tensor("woT", [HPC * DH, HID], F16, kind="ExternalInput")
    cosT = nc.dram_tensor("cosT", [DH, N], F32, kind="ExternalInput")
    sinT = nc.dram_tensor("sinT", [DH, N], F32, kind="ExternalInput")
    outT = nc.dram_tensor("outT", [HID, N], F32, kind="ExternalOutput")

    KT = HID // P  # 16 contraction tiles over hid
    NT = N // P  # 16 token tiles
    NS = N // 512  # 4 token stripes
    MQK = 2 * HPC  # 8 output dim-tiles for q+k

    mult = mybir.AluOpType.mult
    add = mybir.AluOpType.add
    Exp = mybir.ActivationFunctionType.Exp

    with tile.TileContext(nc) as tc:
        with (
            tc.tile_pool(name="const", bufs=1) as const,
            tc.tile_pool(name="persist", bufs=1) as persist,
            tc.tile_pool(name="psum", bufs=3, space="PSUM") as psum,
        ):
            wqk_sb = const.tile([P, KT, 2 * HPC * DH], F16)
            nc.sync.dma_start(
                wqk_sb[:], wqkT[:].rearrange("(kt p) m -> p kt m", p=P)
            )
            wv_sb = const.tile([P, KT, HPC * DH], F16)
            nc.sync.dma_start(wv_sb[:], wvT[:].rearrange("(kt p) m -> p kt m", p=P))
            cos_sb = const.tile([P, N], F32)
            nc.sync.dma_start(cos_sb[:], cosT[:])
            sin_sb = const.tile([P, N], F32)
            nc.sync.dma_start(sin_sb[:], sinT[:])
            ones_sb = const.tile([P, P], F16)
            nc.vector.memset(ones_sb[:], 1.0)

            # persistent intermediates: q/k (rope'd, [dh, tok] per dim-tile),
            # v ([tok, 4*dh]), attention outputs ([dh, tok] per head)
            qk_tiles = [
                persist.tile([P, N], F16, tag=f"qk{m}", name=f"qk{m}") for m in range(MQK)
            ]
            v_sb = persist.tile([P, NT, HPC * DH], F16, tag="v")
            attn_tiles = [
                persist.tile([P, N], F16, tag=f"attn{h}", name=f"attn{h}") for h in range(HPC)
            ]

            # ---------------- Phase A: QKV + RoPE ----------------
            with (
                tc.tile_pool(name="xin", bufs=2) as x_pool,
                tc.tile_pool(name="ropetmp", bufs=2) as tmp_pool,
            ):
                for s in range(NS):
                    x_sb = x_pool.tile([P, KT, 512], F16, tag="x")
                    nc.sync.dma_start(
                        x_sb[:],
                        xT[:, s * 512 : (s + 1) * 512].rearrange(
                            "(kt p) n -> p kt n", p=P
                        ),
                    )
                    sl = slice(s * 512, (s + 1) * 512)
                    for m in range(MQK):
                        ps = psum.tile([P, 512], F32, tag="ps")
                        for kt in range(KT):
                            nc.tensor.matmul(
                                ps[:],
                                wqk_sb[:, kt, m * P : (m + 1) * P],
                                x_sb[:, kt, :],
                                start=(kt == 0),
                                stop=(kt == KT - 1),
                            )
                        # RoPE: out = ps*cos + rot(ps)*sin_signed
                        rot = tmp_pool.tile([P, 512], F32, tag="rot")
                        nc.scalar.copy(rot[0:64, :], ps[64:128, :])
                        nc.scalar.copy(rot[64:128, :], ps[0:64, :])
                        t1 = tmp_pool.tile([P, 512], F32, tag="t1")
                        nc.vector.tensor_tensor(t1[:], rot[:], sin_sb[:, sl], mult)
                        t2 = tmp_pool.tile([P, 512], F32, tag="t2")
                        nc.vector.tensor_tensor(t2[:], ps[:], cos_sb[:, sl], mult)
                        nc.vector.tensor_tensor(
                            qk_tiles[m][:, sl], t1[:], t2[:], add
                        )
                    for tt in range(4):
                        tok = s * 4 + tt
                        psv = psum.tile([P, 512], F32, tag="psv", bufs=2)
                        for kt in range(KT):
                            nc.tensor.matmul(
                                psv[:],
                                x_sb[:, kt, tt * P : (tt + 1) * P],
                                wv_sb[:, kt, :],
                                start=(kt == 0),
                                stop=(kt == KT - 1),
                            )
                        nc.scalar.copy(v_sb[:, tok, :], psv[:])

            # ---------------- Phases B (attention) + C (o-proj) ----------
            with tc.tile_pool(name="late", bufs=3) as late:
                wo_sb = persist.tile([P, HPC, HID], F16, tag="wo")
                nc.sync.dma_start(
                    wo_sb[:], woT[:].rearrange("(ht p) o -> p ht o", p=P)
                )

                for h in range(HPC):
                    kT_tile = qk_tiles[HPC + h]
                    qT_tile = qk_tiles[h]
                    for qs in range(NS):
                        qsl = slice(qs * 512, (qs + 1) * 512)
                        outp = psum.tile([P, 512], F32, tag="psv", bufs=2)
                        den = psum.tile([P, 512], F32, tag="den", bufs=2)
                        for kt in range(NT):
                            st = psum.tile([P, 512], F32, tag="ps")
                            nc.tensor.matmul(
                                st[:],
                                kT_tile[:, kt * P : (kt + 1) * P],
                                qT_tile[:, qsl],
                                start=True,
                                stop=True,
                            )
                            pt = late.tile([P, 512], F16, tag="pt")
                            nc.scalar.activation(pt[:], st[:], Exp, scale=SCALE)
                            nc.tensor.matmul(
                                outp[:],
                                v_sb[:, kt, h * DH : (h + 1) * DH],
                                pt[:],
                                start=(kt == 0),
                                stop=(kt == NT - 1),
                            )
                            nc.tensor.matmul(
                                den[:],
                                ones_sb[:],
                                pt[:],
                                start=(kt == 0),
                                stop=(kt == NT - 1),
                            )
                        rec = late.tile([P, 512], F32, tag="rec")
                        nc.vector.reciprocal(rec[:], den[:])
                        nc.vector.tensor_tensor(
                            attn_tiles[h][:, qsl], outp[:], rec[:], mult
                        )

                for ho in range(HID // P):
                    for ts in range(NS):
                        tsl = slice(ts * 512, (ts + 1) * 512)
                        ps = psum.tile([P, 512], F32, tag="ps")
                        for hi in range(HPC):
                            nc.tensor.matmul(
                                ps[:],
                                wo_sb[:, hi, ho * P : (ho + 1) * P],
                                attn_tiles[hi][:, tsl],
                                start=(hi == 0),
                                stop=(hi == HPC - 1),
                            )
                        ob = late.tile([P, 512], F32, tag="co")
                        nc.scalar.copy(ob[:], ps[:])
                        nc.sync.dma_start(
                            outT[ho * P : (ho + 1) * P, tsl], ob[:]
                        )

    nc.finalize()
    return nc


def get_nc():
    if _NC_CACHE[0] is None:
        _NC_CACHE[0] = build_nc()
    return _NC_CACHE[0]


def make_in_maps(hidden_states, cos, sin, w_qkv, w_o):
    """Build the 8 per-core input maps (host-side shard + transpose + cast)."""
    hidden_states = np.asarray(hidden_states, dtype=np.float32)
    cos = np.asarray(cos, dtype=np.float32)
    sin = np.asarray(sin, dtype=np.float32)
    w_qkv = np.asarray(w_qkv, dtype=np.float32)
    w_o = np.asarray(w_o, dtype=np.float32)

    cosT = np.ascontiguousarray(cos.T)  # [DH, N]
    sinT_signed = np.ascontiguousarray(
        np.concatenate([-sin.T[: DH // 2], sin.T[DH // 2 :]], axis=0)
    )

    xT = [
        np.ascontiguousarray(hidden_states[b].T).astype(np.float16)
        for b in range(B)
    ]

    in_maps = []
    for c in range(8):
        b, g = divmod(c, 4)
        qrows = slice(g * HPC * DH, (g + 1) * HPC * DH)
        krows = slice(HID + g * HPC * DH, HID + (g + 1) * HPC * DH)
        vrows = slice(2 * HID + g * HPC * DH, 2 * HID + (g + 1) * HPC * DH)
        wqkT = (
            np.concatenate([w_qkv[qrows], w_qkv[krows]], axis=0)
            .T.astype(np.float16)
        )
        wvT = w_qkv[vrows].T.astype(np.float16)
        woT = w_o[:, g * HPC * DH : (g + 1) * HPC * DH].T.astype(np.float16)
        in_maps.append(
            {
                "xT": xT[b],
                "wqkT": np.ascontiguousarray(wqkT),
                "wvT": np.ascontiguousarray(wvT),
                "woT": np.ascontiguousarray(woT),
                "cosT": cosT,
                "sinT": sinT_signed,
            }
        )
    return in_maps


def assemble_output(results):
    """Sum the 4 o-proj partials per batch and transpose back."""
    out = np.zeros((B, N, HID), dtype=np.float32)
    for c, res in enumerate(results):
        b = c // 4
        out[b] += res["outT"].T
    return out


def kernel(hidden_states, cos, sin, w_qkv, w_o):
    from concourse.bass_utils import run_bass_kernel_spmd

    nc = get_nc()
    in_maps = make_in_maps(hidden_states, cos, sin, w_qkv, w_o)
    res = run_bass_kernel_spmd(nc, in_maps, core_ids=list(range(8)))
    return assemble_output(res.results)
